# revision 9
# baseline (speedup 1.0000x reference)
"""Trainium2 Bass kernel for nn_Attention_23476291240422 (sparse attention:
causal + 128-wide noncausal prefix block; b=4, n=2048, dim=2048, 16 heads,
d=128) distributed across 8 NeuronCores.

Sharding: head-parallel (2 heads/core) QKV projection + attention, AllToAll
(bf16) to flip head-sharded -> row-sharded, row-parallel out-proj (1024
rows/core). bf16 compute with fp32 PSUM accumulation.

Schedule: stage 1 (QKV projection) interleaved per batch with phase-A
attention (i-chunks {1,3}); AllToAll-A; phase-B attention (i-chunks {0,2})
head-outer, exp-bound, with the AllToAll-B split into two per-head
collectives; out-proj pass-0 follows (covering AllToAll-B), then pass-1.
Attention output is transposed on TensorE and shipped column-major so all
out-proj lhs loads are contiguous; a2a writes are fused per (head, chunk).
A dummy collective absorbs the CC-engine start delay and dummy matmuls warm
the PE clock gate during the first weight DMAs. One shared 4-buffer PSUM
pool serves projection/dots/out-proj chains so attention PV and transposes
keep their own double-buffered banks.

kernel(x, w_qkv, w_out, b_out) -> [4, 2048, 2048] float32.
"""
import os
import sys
import types

import numpy as np
import ml_dtypes

import concourse.bass as bass
import concourse.mybir as mybir
import concourse.tile as tile
from concourse import bacc
from concourse import bass_utils

B, N, DIM = 4, 2048, 2048
HEADS, D, L = 16, 128, 128
W = 8
HPC = HEADS // W          # 2 heads per core
ROWS = B * N              # 8192
RPC = ROWS // W           # 1024 rows per core
SCALE = float(D) ** -0.5
P = 128
KT = DIM // P             # 16
S1CH = 512                # stage-1 seq chunk width
NBC = N // S1CH           # 4 stage-1 chunks per batch
CHW = 512                 # stage-2 i-chunk width / stage-4 col chunk
NJT = N // P              # 16 j-tiles
F32, BF16 = mybir.dt.float32, mybir.dt.bfloat16
PHASE_CHUNKS = ([1, 3], [0, 2])
M_BASE = (4, 0)  # stage-4 row-tile base per phase


def _install_ntff_hook():
    try:
        import antenv.axon_hooks  # noqa: F401
        return
    except ImportError:
        pass
    try:
        import antenv
        from trn_agent_boot.trn_boot import _ntff_profile_via_ctypes
        hook = [_ntff_profile_via_ctypes("/opt/axon/libaxon_pjrt.so")]
        mod = types.ModuleType("antenv.axon_hooks")
        mod.get_axon_ntff_profile_hook = lambda: hook[0]
        mod.set_axon_ntff_profile_hook = lambda h: hook.__setitem__(0, h)
        sys.modules["antenv.axon_hooks"] = mod
        antenv.axon_hooks = mod
    except Exception:
        pass


def build():
    nc = bacc.Bacc("TRN2", target_bir_lowering=False, debug=False, num_devices=W)

    xT = nc.dram_tensor("xT", [DIM, ROWS], BF16, kind="ExternalInput")
    wq = nc.dram_tensor("wq", [DIM, 6 * P], BF16, kind="ExternalInput")  # q0 q1 k0 k1 v0 v1
    wout = nc.dram_tensor("wout", [DIM, DIM], BF16, kind="ExternalInput")
    bout = nc.dram_tensor("bout", [1, DIM], F32, kind="ExternalInput")
    out = nc.dram_tensor("out", [RPC, DIM], F32, kind="ExternalOutput")

    tri_np = (np.arange(P)[:, None] <= np.arange(P)[None, :]).astype(ml_dtypes.bfloat16)
    tri = nc.inline_tensor(tri_np, name="tri")
    ident = nc.inline_tensor(np.eye(P, dtype=ml_dtypes.bfloat16), name="ident")

    def s1_batch(b, wq_bf, qk_b, v_dram_b, s1xf, mmps, s1v_pool, xb0=None):
        xT_r = xT.rearrange("(kt p) n -> p kt n", p=P)
        for c in range(NBC):
            seq0 = b * N + c * S1CH
            if b == 0 and c == 0 and xb0 is not None:
                xb = xb0
            else:
                xb = s1xf.tile([P, KT, S1CH], BF16, tag="xb")
                nc.sync.dma_start(xb[:], xT_r[:, :, seq0:seq0 + S1CH])
            for m in range(4):
                ps = mmps.tile([P, CHW], F32, tag="mm")
                for kt in range(KT):
                    nc.tensor.matmul(
                        ps[:, :S1CH], wq_bf[:, kt, m * P:(m + 1) * P], xb[:, kt],
                        start=(kt == 0), stop=(kt == KT - 1))
                nc.vector.tensor_copy(
                    qk_b[:, m, c * S1CH:(c + 1) * S1CH], ps[:, :S1CH])
            for st2 in range(S1CH // P):
                st = c * (S1CH // P) + st2  # seq-tile within batch (0..15)
                psv = mmps.tile([P, CHW], F32, tag="mm")
                for kt in range(KT):
                    nc.tensor.matmul(
                        psv[:, :HPC * P], xb[:, kt, st2 * P:(st2 + 1) * P],
                        wq_bf[:, kt, 4 * P:6 * P],
                        start=(kt == 0), stop=(kt == KT - 1))
                vst = s1v_pool.tile([P, HPC * P], BF16, tag="vst")
                nc.vector.tensor_copy(vst[:], psv[:, :HPC * P])
                nc.sync.dma_start(v_dram_b[st * P:(st + 1) * P, :], vst[:])

    def s2_attn(b, hl, chunks, qk_b, v_dram_b, a2a_target, pools,
                tri_sb, ident_sb, va_pre=None):
        """Attention for one (batch, head) over the given i-chunks.
        a2a_target: [W, P, RPC//2] AP; chunk writes [dest, :, :] col-major."""
        s2va, s2pt, s2sm, mmps, s2att, s2tp = pools
        if va_pre is not None:
            va = va_pre
        else:
            va = s2va.tile([P, NJT, P + 1], BF16, tag="va")
            nc.vector.memset(va[:, :, P:P + 1], 1.0)
            v_r = v_dram_b.rearrange("(jt p) d -> p jt d", p=P)
            nc.sync.dma_start(va[:, :, :P], v_r[:, :, hl * P:(hl + 1) * P])
        qT = qk_b[:, hl]
        kTt = qk_b[:, 2 + hl]
        last_w = None
        # all dots/exp first so the later chunk's dots hide the earlier
        # chunk's exp latency ahead of its PV chains
        pts = {}
        for c in chunks:
            pt = s2pt.tile([P, NJT, CHW], BF16, tag="pt")
            pts[c] = pt
            for J in range(4 * c + 4):
                k_off = max(0, J - 4 * c)
                nn_ = CHW - P * k_off
                i0 = c * CHW + P * k_off
                pss = mmps.tile([P, CHW], F32, tag="mm")
                nc.tensor.matmul(
                    pss[:, :nn_], kTt[:, J * P:(J + 1) * P],
                    qT[:, i0:(c + 1) * CHW], start=True, stop=True)
                nc.scalar.activation(
                    pt[:, J, P * k_off:], pss[:, :nn_],
                    mybir.ActivationFunctionType.Exp, scale=SCALE)
                if J >= 4 * c and not (c == 0 and J == 0):
                    nc.vector.tensor_mul(
                        pt[:, J, P * k_off:P * (k_off + 1)],
                        pt[:, J, P * k_off:P * (k_off + 1)], tri_sb[:])
        for c in chunks:
            pt = pts[c]
            attns = []
            for pp in range(4):
                it = 4 * c + pp
                att = s2att.tile([P, P + 1], F32, tag="att")
                for J in range(it + 1):
                    nc.tensor.matmul(
                        att[:], pt[:, J, P * pp:P * (pp + 1)], va[:, J],
                        start=(J == 0), stop=(J == it))
                recip = s2sm.tile([P, 1], F32, tag="recip")
                nc.vector.reciprocal(recip[:], att[:, P:P + 1])
                attn = s2sm.tile([P, P], BF16, tag="attn", bufs=4)
                nc.vector.tensor_scalar_mul(attn[:], att[:, :P], recip[:])
                attns.append(attn)
            # deferred transposes + single contiguous a2a write per chunk
            attnT_c = s2sm.tile([P, 4, P], BF16, tag="attnT", bufs=2)
            for pp in range(4):
                attT_ps = s2tp.tile([P, P], BF16, tag="attTps")
                nc.tensor.transpose(attT_ps[:], attns[pp][:], ident_sb[:])
                nc.vector.tensor_copy(attnT_c[:, pp], attT_ps[:])
            dest = b * 2 + c // 2
            last_w = nc.sync.dma_start(
                a2a_target[dest].rearrange("p (pp i) -> p pp i", pp=4),
                attnT_c[:])
        return last_w

    def preload_va(b, hl, v_dram_b, s2va):
        va = s2va.tile([P, NJT, P + 1], BF16, tag="va")
        nc.vector.memset(va[:, :, P:P + 1], 1.0)
        v_r = v_dram_b.rearrange("(jt p) d -> p jt d", p=P)
        nc.sync.dma_start(va[:, :, :P], v_r[:, :, hl * P:(hl + 1) * P])
        return va

    with tile.TileContext(nc) as tc:
        with (
            tc.tile_pool(name="persist", bufs=1) as persist,
            tc.tile_pool(name="dram", bufs=1, space="DRAM") as dram,
        ):
            tri_sb = persist.tile([P, P], BF16)
            ident_sb = persist.tile([P, P], BF16)
            bout_sb = persist.tile([P, DIM], F32)

            v_drams = [dram.tile([N, HPC * P], BF16, name=f"v_dram{b}")
                       for b in range(B)]
            # a2a payload col-major: [dest, head, d, rows]
            a2a_inA = dram.tile([W, HPC, P, RPC // 2], BF16, name="a2a_inA")
            a2a_outA = dram.tile([W, HPC, P, RPC // 2], BF16, name="a2a_outA")
            a2a_inB = [dram.tile([W, P, RPC // 2], BF16, name=f"a2a_inB{h}")
                       for h in range(HPC)]
            a2a_outB = [dram.tile([W, P, RPC // 2], BF16, name=f"a2a_outB{h}")
                        for h in range(HPC)]
            cc_warm_in = dram.tile([W, P, 16], BF16, name="cc_warm_in")
            cc_warm_out = dram.tile([W, P, 16], BF16, name="cc_warm_out")

            woutb_r = wout.rearrange("(kt p) c -> p kt c", p=P)

            with (
                tc.tile_pool(name="qk", bufs=1) as qkpool,
                tc.tile_pool(name="s2va", bufs=5) as s2va,
                tc.tile_pool(name="s2pt", bufs=2) as s2pt,
                tc.tile_pool(name="s2sm", bufs=12) as s2sm,
                tc.tile_pool(name="mmps", bufs=4, space="PSUM") as mmps,
                tc.tile_pool(name="s2att", bufs=2, space="PSUM") as s2att,
                tc.tile_pool(name="s2tp", bufs=2, space="PSUM") as s2tp,
            ):
                qk_bs = [qkpool.tile([P, 4, N], BF16, name=f"qkb{b}")
                         for b in range(B)]
                s2pools = (s2va, s2pt, s2sm, mmps, s2att, s2tp)

                with (
                    tc.tile_pool(name="s1w", bufs=1) as s1w,
                    tc.tile_pool(name="s1v", bufs=3) as s1v_pool,
                    tc.tile_pool(name="s1xf", bufs=3) as s1xf,
                ):
                    wq_bf = s1w.tile([P, KT, 6 * P], BF16)
                    wq_r = wq.rearrange("(kt p) c -> p kt c", p=P)
                    # startup: tiny ident first (feeds PE warm-up), then the
                    # first weight/x groups so the first qk chain starts ASAP.
                    nc.sync.dma_start(ident_sb[:], ident.ap())
                    nc.sync.dma_start(wq_bf[:, 0:4], wq_r[:, 0:4])
                    xb0 = s1xf.tile([P, KT, S1CH], BF16, tag="xb")
                    xT_r0 = xT.rearrange("(kt p) n -> p kt n", p=P)
                    nc.sync.dma_start(xb0[:, 0:4], xT_r0[:, 0:4, 0:S1CH])
                    nc.sync.dma_start(tri_sb[:], tri.ap())
                    # warm the CC engine: a tiny collective absorbs the
                    # one-time collective start delay off the critical path
                    ccw = s2sm.tile([P, W * 16], BF16, tag="ccw", bufs=1)
                    nc.vector.memset(ccw[:], 0.0)
                    nc.gpsimd.dma_start(
                        cc_warm_in.rearrange("w p c -> p w c"),
                        ccw[:].rearrange("p (w c) -> p w c", w=W))
                    nc.gpsimd.collective_compute(
                        "AllToAll", mybir.AluOpType.bypass,
                        replica_groups=[list(range(W))],
                        ins=[cc_warm_in[:].opt()], outs=[cc_warm_out[:].opt()],
                    )
                    # PE warm-up: opens the HAM clock gate during DMA wait
                    for _wi in range(20):
                        wps = mmps.tile([P, CHW], F32, tag="mm")
                        nc.tensor.matmul(wps[:, :P], ident_sb[:], ident_sb[:],
                                         start=True, stop=True)
                    for kq in range(1, 4):
                        nc.sync.dma_start(
                            xb0[:, 4 * kq:4 * (kq + 1)],
                            xT_r0[:, 4 * kq:4 * (kq + 1), 0:S1CH])
                        nc.sync.dma_start(
                            wq_bf[:, 4 * kq:4 * (kq + 1)],
                            wq_r[:, 4 * kq:4 * (kq + 1)])

                    # interleave: s1(b) then phase-A attention of batch b
                    for b in range(B):
                        s1_batch(b, wq_bf, qk_bs[b], v_drams[b],
                                 s1xf, mmps, s1v_pool,
                                 xb0=xb0 if b == 0 else None)
                        for hl in range(HPC):
                            s2_attn(b, hl, PHASE_CHUNKS[0], qk_bs[b],
                                    v_drams[b], a2a_inA[:, hl], s2pools,
                                    tri_sb, ident_sb)

                nc.gpsimd.collective_compute(
                    "AllToAll", mybir.AluOpType.bypass,
                    replica_groups=[list(range(W))],
                    ins=[a2a_inA[:].opt()], outs=[a2a_outA[:].opt()],
                )

                with (
                    tc.tile_pool(name="s4l", bufs=2) as s4l,
                    tc.tile_pool(name="s4w", bufs=2) as s4w,
                    tc.tile_pool(name="s4o", bufs=4) as s4o,
                ):
                    # wb0/wb1/bias have no collective deps: load early.
                    wb0 = s4w.tile([P, KT, CHW], BF16, tag="wb")
                    nc.sync.dma_start(wb0[:], woutb_r[:, :, 0:CHW])
                    nc.sync.dma_start(
                        bout_sb[:], bout.ap().to_broadcast((P, DIM)))
                    wb1 = s4w.tile([P, KT, CHW], BF16, tag="wb")
                    nc.sync.dma_start(wb1[:], woutb_r[:, :, CHW:2 * CHW])

                    def s4_chunk(phase, lhs, wb, ncx):
                        for ml in range(4):
                            m = M_BASE[phase] + ml
                            ps4 = mmps.tile([P, CHW], F32, tag="mm")
                            for kt in range(KT):
                                nc.tensor.matmul(
                                    ps4[:],
                                    lhs[:, kt, ml * P:(ml + 1) * P],
                                    wb[:, kt],
                                    start=(kt == 0), stop=(kt == KT - 1))
                            osb = s4o.tile([P, CHW], F32, tag="osb")
                            nc.vector.tensor_tensor(
                                osb[:], ps4[:],
                                bout_sb[:, ncx * CHW:(ncx + 1) * CHW],
                                mybir.AluOpType.add)
                            nc.sync.dma_start(
                                out[m * P:(m + 1) * P,
                                    ncx * CHW:(ncx + 1) * CHW], osb[:])

                    # ---- phase B, head 0 of each batch ----
                    for b in range(B):
                        s2_attn(b, 0, PHASE_CHUNKS[1], qk_bs[b], v_drams[b],
                                a2a_inB[0], s2pools, tri_sb, ident_sb)
                    nc.gpsimd.collective_compute(
                        "AllToAll", mybir.AluOpType.bypass,
                        replica_groups=[list(range(W))],
                        ins=[a2a_inB[0][:].opt()], outs=[a2a_outB[0][:].opt()],
                    )
                    # ---- phase B, head 1 ----
                    vas1 = [preload_va(b, 1, v_drams[b], s2va)
                            for b in range(B)]
                    s2_attn(0, 1, PHASE_CHUNKS[1], qk_bs[0], v_drams[0],
                            a2a_inB[1], s2pools, tri_sb, ident_sb,
                            va_pre=vas1[0])
                    # lhs0 (needs AllToAll-A): contiguous per-source loads
                    lhs0 = s4l.tile([P, KT, RPC // 2], BF16, tag="lhs")
                    for w_src in range(W):
                        nc.sync.dma_start(
                            lhs0[:, 2 * w_src:2 * w_src + 2],
                            a2a_outA[w_src].rearrange("h p r -> p h r"))
                    for b in range(1, B):
                        s2_attn(b, 1, PHASE_CHUNKS[1], qk_bs[b], v_drams[b],
                                a2a_inB[1], s2pools, tri_sb, ident_sb,
                                va_pre=vas1[b])
                    nc.gpsimd.collective_compute(
                        "AllToAll", mybir.AluOpType.bypass,
                        replica_groups=[list(range(W))],
                        ins=[a2a_inB[1][:].opt()], outs=[a2a_outB[1][:].opt()],
                    )
                    # ---- pass 0 ----
                    s4_chunk(0, lhs0, wb0, 0)
                    s4_chunk(0, lhs0, wb1, 1)
                    wb2 = s4w.tile([P, KT, CHW], BF16, tag="wb")
                    nc.sync.dma_start(wb2[:], woutb_r[:, :, 2 * CHW:3 * CHW])
                    wb3 = s4w.tile([P, KT, CHW], BF16, tag="wb")
                    nc.sync.dma_start(wb3[:], woutb_r[:, :, 3 * CHW:4 * CHW])
                    # lhs1 loads: per-head halves gated on their own
                    # collective; pass-1 then starts fully resident.
                    lhs1 = s4l.tile([P, KT, RPC // 2], BF16, tag="lhs")
                    for w_src in range(W):
                        nc.sync.dma_start(
                            lhs1[:, 2 * w_src], a2a_outB[0][w_src])
                    for w_src in range(W):
                        nc.sync.dma_start(
                            lhs1[:, 2 * w_src + 1], a2a_outB[1][w_src])
                    s4_chunk(0, lhs0, wb2, 2)
                    s4_chunk(0, lhs0, wb3, 3)
                    # ---- pass 1: reuse wb2/wb3 first, reload 0/1 last ----
                    s4_chunk(1, lhs1, wb2, 2)
                    s4_chunk(1, lhs1, wb3, 3)
                    wbp0 = s4w.tile([P, KT, CHW], BF16, tag="wb")
                    nc.sync.dma_start(wbp0[:], woutb_r[:, :, 0:CHW])
                    s4_chunk(1, lhs1, wbp0, 0)
                    wbp1 = s4w.tile([P, KT, CHW], BF16, tag="wb")
                    nc.sync.dma_start(wbp1[:], woutb_r[:, :, CHW:2 * CHW])
                    s4_chunk(1, lhs1, wbp1, 1)

    nc.compile()
    return nc


_NC = None


def _get_nc():
    global _NC
    if _NC is None:
        _NC = build()
    return _NC


last_exec_time_ns = None
last_results = None


def kernel(x, w_qkv, w_out, b_out):
    global last_exec_time_ns, last_results
    _install_ntff_hook()
    nc = _get_nc()

    x = np.asarray(x, dtype=np.float32)
    w_qkv = np.asarray(w_qkv, dtype=np.float32)
    w_out = np.asarray(w_out, dtype=np.float32)
    b_out = np.asarray(b_out, dtype=np.float32)

    bf = ml_dtypes.bfloat16
    xT = np.ascontiguousarray(x.reshape(ROWS, DIM).T.astype(bf))
    wout_b = np.ascontiguousarray(w_out.astype(bf))
    bout2 = np.ascontiguousarray(b_out.reshape(1, DIM))

    in_maps = []
    for core in range(W):
        cols = [w_qkv[:, part * (HEADS * D) + core * HPC * D:
                      part * (HEADS * D) + (core + 1) * HPC * D]
                for part in range(3)]
        wq_c = np.ascontiguousarray(np.concatenate(cols, axis=1).astype(bf))
        in_maps.append({"xT": xT, "wq": wq_c, "wout": wout_b, "bout": bout2})

    trace = os.environ.get("KERNEL_TRACE", "") not in ("", "0")
    res = bass_utils.run_bass_kernel_spmd(
        nc, in_maps, core_ids=list(range(W)), trace=trace)
    last_exec_time_ns = res.exec_time_ns
    last_results = res

    out = np.concatenate([res.results[c]["out"] for c in range(W)], axis=0)
    return np.ascontiguousarray(out.reshape(B, N, DIM), dtype=np.float32)
